# revision 29
# baseline (speedup 1.0000x reference)
"""ClassAttention Trainium2 kernel (Bass/Tile), data-parallel over batch on 8 cores.

Math (per batch b):
  q = x[b,0] @ W_q                      -> [H, D]
  k = x[b] @ W_k ; v = x[b] @ W_v       (W_k/W_v = halves of W_kv)
  scores = (q * SCALE) . k  per head    -> [H, N]
  attn = softmax(scores, axis=N)
  cls = attn @ v (per head)             -> [H*D]
  out[b] = cls @ W_proj + b_proj

Algebraic tricks eliminate both giant matmuls (x@W_k and x@W_v):
 1. Fold q into the weights so k is never materialized:
      Q'_b[64h+d, h] = q_b[h,d] * SCALE   (block-diagonal scatter, [C, H])
      G_b = W_k @ Q'_b                    ([C, H], per batch)
      scores^T = G_b^T @ x_b^T
 2. Reassociate the value path: cls = (attn @ x) @ W_v
      y_b = attn_b @ x_b                  ([H, C], contraction over tokens)
      cls  = diag-blocks of (y @ W_v)     (one 128-row matmul for all batches)

All layout work happens on the HOST: x is passed twice (natural and
transposed), both bf16 and pre-swizzled so every DMA is a plain
contiguous copy with 16KB runs per partition. Weights are pre-cast to
bf16 and pre-swizzled too (W_k additionally pre-transposed), so the
device does zero transposes or casts of its inputs. On-chip token index
j = 8p + g (partition p, group g). All matmuls bf16 with fp32
accumulation. Each core handles 8 batches; no collectives.
"""

import numpy as np
from contextlib import ExitStack

B, N, C = 64, 1024, 1024
H, D = 16, 64
SCALE = D**-0.5
NCORES = 8
BL = B // NCORES  # batches per core
CCH = C // 128  # chunks over any 1024-dim
GT = N // 128  # token groups per batch

_BUILT = {}


def _build_module():
    import concourse.mybir as mybir
    import concourse.tile as tile
    from concourse import bacc
    from concourse.masks import make_identity

    f32 = mybir.dt.float32
    bf16 = mybir.dt.bfloat16
    AF = mybir.ActivationFunctionType

    nc = bacc.Bacc("TRN2", target_bir_lowering=False, debug=False)

    x_d = nc.dram_tensor("x_nat", [BL, N, C], bf16, kind="ExternalInput")
    xt_d = nc.dram_tensor("x_tr", [BL, 128, CCH, N], bf16, kind="ExternalInput")
    xclsT_d = nc.dram_tensor("xclsT", [128, CCH, BL], bf16, kind="ExternalInput")
    wkT_d = nc.dram_tensor("wkT", [128, CCH, C], bf16, kind="ExternalInput")
    wv_d = nc.dram_tensor("wv", [128, CCH, H * D], bf16, kind="ExternalInput")
    wq_d = nc.dram_tensor("wq", [128, CCH, H * D], bf16, kind="ExternalInput")
    wp_d = nc.dram_tensor("wp", [128, CCH, C], bf16, kind="ExternalInput")
    bp_d = nc.dram_tensor("b_proj", [1, C], bf16, kind="ExternalInput")
    out_d = nc.dram_tensor("out", [BL, C], f32, kind="ExternalOutput")

    with tile.TileContext(nc) as tc, ExitStack() as ctx:
        const = ctx.enter_context(tc.tile_pool(name="const", bufs=1))
        work = ctx.enter_context(tc.tile_pool(name="work", bufs=2))
        xpool = ctx.enter_context(tc.tile_pool(name="xp", bufs=3))
        xtpool = ctx.enter_context(tc.tile_pool(name="xtp", bufs=3))
        apool = ctx.enter_context(tc.tile_pool(name="ap", bufs=18))
        ps_t = ctx.enter_context(tc.tile_pool(name="ps_t", bufs=2, space="PSUM"))
        ps_acc = ctx.enter_context(tc.tile_pool(name="ps_acc", bufs=6, space="PSUM"))

        # DMA split across both queues so the first weights land fast:
        #   gpsimd (SWDGE): xclsT, wq, x0..x7, wp       (~21 MB)
        #   sync   (HWDGE): b, wkT, xt0..xt7, wv, out   (~21 MB)
        # Issue the first DMA triggers before any compute-engine setup so the
        # queues start streaming immediately.
        xclsT = const.tile([128, CCH, BL], bf16, tag="xclsT")
        nc.gpsimd.dma_start(out=xclsT[:, :, :], in_=xclsT_d[:, :, :])
        wq_sb = const.tile([128, CCH, H * D], bf16, tag="wq")
        nc.gpsimd.dma_start(out=wq_sb[:, :, :], in_=wq_d[:, :, :])
        b_sb = const.tile([1, C], bf16, tag="b")
        nc.sync.dma_start(out=b_sb[:, :], in_=bp_d[:, :])
        wkT_sb = const.tile([128, CCH, C], bf16, tag="wkT")
        nc.sync.dma_start(out=wkT_sb[:, :, :], in_=wkT_d[:, :, :])

        # x/xt arrive in half-tiles so compute can chase the stream at finer
        # granularity (scores/y consume chunks in ascending order).
        def load_x(b):
            x_sb = xpool.tile([128, GT, C], bf16, tag="x")
            xv = x_d[b, :, :].rearrange("(p g) c -> p g c", g=GT)
            nc.gpsimd.dma_start(out=x_sb[:, 0 : GT // 2, :], in_=xv[:, 0 : GT // 2, :])
            nc.gpsimd.dma_start(out=x_sb[:, GT // 2 :, :], in_=xv[:, GT // 2 :, :])
            return x_sb

        def load_xt(b):
            xt_sb = xtpool.tile([128, CCH, N], bf16, tag="xt")
            nc.sync.dma_start(
                out=xt_sb[:, 0 : CCH // 2, :], in_=xt_d[b, :, 0 : CCH // 2, :]
            )
            nc.sync.dma_start(out=xt_sb[:, CCH // 2 :, :], in_=xt_d[b, :, CCH // 2 :, :])
            return xt_sb

        x_tiles = {b: load_x(b) for b in range(2)}
        xt_tiles = {b: load_xt(b) for b in range(2)}

        # ---------------- constants ----------------
        ident_bf = const.tile([128, 128], bf16, tag="ident_bf")
        make_identity(nc, ident_bf[:, :])
        ones_sb = const.tile([1, BL], bf16, tag="ones")
        nc.vector.memset(ones_sb[:, :], 1.0)

        # PE p-state warmup sized to the measured ~20us wq-arrival window:
        # the clock reaches 2.4 GHz only after 3us of continuous execution,
        # so bridge the idle weight-load wait with dummy matmuls and enter
        # the q/G preamble at full clock, gap-free.
        warm_ps = ps_t.tile([128, 128], f32, tag="ps_tr", name="warm_ps")
        for _ in range(380):
            nc.tensor.matmul(warm_ps[:, :], ident_bf[:, :], ident_bf[:, :])

        # ---------------- q for all batches (wide form) ----------------
        # psq[b, m] = sum_c xcls[b, c] W_q[c, m]
        psq = [ps_acc.tile([BL, 512], f32, tag="ps_acc", name=f"psq{i}") for i in range(2)]
        for cc in range(CCH):
            for half in range(2):
                nc.tensor.matmul(
                    psq[half][:, :],
                    xclsT[:, cc, :],
                    wq_sb[:, cc, half * 512 : (half + 1) * 512],
                    start=(cc == 0),
                    stop=(cc == CCH - 1),
                )
        qn = work.tile([BL, C], bf16, tag="scr")
        for half in range(2):
            nc.vector.tensor_copy(qn[:, half * 512 : (half + 1) * 512], psq[half][:, :])

        # scatter q into block-diagonal Q' (SCALE folded): Q'[p(j), jc, b*H+h]
        qp_sb = const.tile([128, CCH, BL * H], bf16, tag="qp")
        nc.vector.memset(qp_sb[:, :, :], 0.0)
        for m in range(CCH):
            psqt = ps_t.tile([128, BL], f32, tag="ps_tr")
            nc.tensor.matmul(
                psqt[:, :], qn[:, m * 128 : (m + 1) * 128], ident_bf[0:BL, 0:BL]
            )
            # head of c' = 128*m + p is 2m + p//64
            qv = qp_sb[:, m, :].rearrange("p (b h) -> p h b", h=H)
            nc.scalar.activation(qv[0:64, 2 * m, :], psqt[0:64, :], AF.Copy, scale=SCALE)
            nc.scalar.activation(
                qv[64:128, 2 * m + 1, :], psqt[64:128, :], AF.Copy, scale=SCALE
            )

        # ---------------- G^T = Q'^T @ W_k^T  ([B*H, C], all batches) -------
        gt_ps = [ps_acc.tile([128, 512], f32, tag="ps_acc", name=f"gt_ps{i}") for i in range(2)]
        for jc in range(CCH):
            for half in range(2):
                nc.tensor.matmul(
                    gt_ps[half][:, :],
                    qp_sb[:, jc, :],
                    wkT_sb[:, jc, half * 512 : (half + 1) * 512],
                    start=(jc == 0),
                    stop=(jc == CCH - 1),
                )
        gt_sb = work.tile([128, C], bf16, tag="scr", name="gt_sb")
        for half in range(2):
            nc.vector.tensor_copy(
                gt_sb[:, half * 512 : (half + 1) * 512], gt_ps[half][:, :]
            )
        # transpose to G: g_sb[p(c), cc, b*H+h]
        g_sb = const.tile([128, CCH, BL * H], bf16, tag="g")
        for cb in range(CCH):
            pst = ps_t.tile([128, 128], f32, tag="ps_tr")
            nc.tensor.matmul(
                pst[:, :], gt_sb[:, cb * 128 : (cb + 1) * 128], ident_bf[:, :]
            )
            if cb % 2 == 0:
                nc.vector.tensor_copy(g_sb[:, cb, :], pst[:, :])
            else:
                nc.scalar.copy(g_sb[:, cb, :], pst[:, :])

        # y rows for all batches, assembled at partition b*H+h: [128, C]
        yn_all = const.tile([128, C], bf16, tag="yn_all")
        # y^T for all batches: [p(c), cc, b*H+h]
        yT_all = const.tile([128, CCH, BL * H], bf16, tag="yT")
        yn_pending = {}

        def flush_yn(b):
            # deferred one iteration so the scalar-ring DMA trigger never
            # waits (yn copies are long done), avoiding head-of-line blocking
            # of the exp that shares the scalar engine
            yn = yn_pending.pop(b, None)
            if yn is not None:
                nc.scalar.dma_start(out=yn_all[b * H : (b + 1) * H, :], in_=yn[:, :])

        # ---------------- main loop: software-pipelined ----------------
        # Tensor stream per iteration:
        #   scores_mm(b+1) | attn_tr(b) | y(b) | yT(b)
        # with softmax(b+1) issued on vector/scalar between attn_tr(b) and
        # y(b), so the PE never waits on the softmax serial chain (which
        # would also drop its p-state clock).

        def scores_mm(b):
            """scores^T = G_b^T @ x^T : accumulate into 2 PSUM halves."""
            xt = xt_tiles.pop(b)
            if b + 2 < BL:
                xt_tiles[b + 2] = load_xt(b + 2)
            ps_s = [
                ps_acc.tile([H, 512], f32, tag="ps_acc", name=f"ps_s{i}")
                for i in range(2)
            ]
            for cc in range(CCH):
                for half in range(2):
                    nc.tensor.matmul(
                        ps_s[half][:, :],
                        g_sb[:, cc, b * H : (b + 1) * H],
                        xt[:, cc, half * 512 : (half + 1) * 512],
                        start=(cc == 0),
                        stop=(cc == CCH - 1),
                    )
            return ps_s

        def softmax(ps_s):
            """PSUM scores -> attnT bf16 [H, N] (vector/scalar only).

            Scores are O(1) here (q,k unit-variance, 1/sqrt(D) scale), so the
            usual max-subtraction is skipped: exp reads straight from PSUM
            (fp32 exp overflows only past ~88), halving the serial chain.
            """
            sT_exp = work.tile([H, N], bf16, tag="sTexp")
            sume = []
            for half in range(2):
                acc = work.tile([H, 1], f32, tag=f"sume{half}", name=f"sume{half}")
                nc.scalar.activation(
                    sT_exp[:, half * 512 : (half + 1) * 512],
                    ps_s[half][:, :],
                    AF.Exp,
                    accum_out=acc[:, :],
                )
                sume.append(acc)
            ssum = work.tile([H, 1], f32, tag="ssum")
            nc.vector.tensor_add(ssum[:, :], sume[0][:, :], sume[1][:, :])
            rs = work.tile([H, 1], f32, tag="rs")
            nc.vector.reciprocal(rs[:, :], ssum[:, :])
            attnT = work.tile([H, N], bf16, tag="attnT")
            # out view visits memory g*128+p while reading token 8p+g, so
            # block g of attnT memory holds exactly token-group g's weights
            # in partition order (contiguous stationaries for the transposes)
            nc.vector.tensor_scalar_mul(
                attnT[:, :].rearrange("h (g p) -> h p g", g=GT),
                sT_exp[:, :].rearrange("h (p g) -> h p g", g=GT),
                rs[:, :],
            )
            return attnT

        def attn_tr(attnT):
            """Transpose attn to per-token-group tiles [128, H]."""
            attn_tiles = []
            for g in range(GT):
                ps_a = ps_t.tile([128, H], f32, tag="ps_tr")
                nc.tensor.matmul(
                    ps_a[:, :], attnT[:, g * 128 : (g + 1) * 128], ident_bf[0:H, 0:H]
                )
                a_sb = apool.tile([128, H], bf16, tag="attn")
                nc.vector.tensor_copy(a_sb[:, :], ps_a[:, :])
                attn_tiles.append(a_sb)
            return attn_tiles

        def y_and_yT(b, attn_tiles):
            """y_b = attn_b @ x_b -> yT_all columns."""
            x_sb = x_tiles.pop(b)
            if b + 2 < BL:
                x_tiles[b + 2] = load_x(b + 2)
            ps_y = [
                ps_acc.tile([H, 512], f32, tag="ps_acc", name=f"ps_y{i}")
                for i in range(2)
            ]
            for g in range(GT):
                for half in range(2):
                    nc.tensor.matmul(
                        ps_y[half][:, :],
                        attn_tiles[g][:, :],
                        x_sb[:, g, half * 512 : (half + 1) * 512],
                        start=(g == 0),
                        stop=(g == GT - 1),
                    )
            yn = work.tile([H, C], bf16, tag="yn")
            for half in range(2):
                nc.vector.tensor_copy(
                    yn[:, half * 512 : (half + 1) * 512], ps_y[half][:, :]
                )
            yn_pending[b] = yn

        attnT_cur = softmax(scores_mm(0))
        for b in range(BL):
            flush_yn(b - 1)
            ps_s_next = scores_mm(b + 1) if b + 1 < BL else None
            tiles_b = attn_tr(attnT_cur)
            attnT_cur = softmax(ps_s_next) if ps_s_next is not None else None
            y_and_yT(b, tiles_b)
        flush_yn(BL - 1)

        # late weight loads on the sync queue (needed only below)
        wv_sb = const.tile([128, CCH, H * D], bf16, tag="wv")
        nc.sync.dma_start(out=wv_sb[:, :, :], in_=wv_d[:, :, :])
        wp_sb = const.tile([128, CCH, C], bf16, tag="wp")
        nc.sync.dma_start(out=wp_sb[:, :, :], in_=wp_d[:, :, :])

        # transpose yn_all into yT_all: [p(c), cc, b*H+h]
        for cc in range(CCH):
            ps_yt = ps_t.tile([128, 128], f32, tag="ps_tr", name="ps_yt_end")
            nc.tensor.matmul(
                ps_yt[:, :], yn_all[:, cc * 128 : (cc + 1) * 128], ident_bf[:, :]
            )
            if cc % 2 == 0:
                nc.vector.tensor_copy(yT_all[:, cc, :], ps_yt[:, :])
            else:
                nc.scalar.copy(yT_all[:, cc, :], ps_yt[:, :])

        # ---------------- cls_nat = y @ W_v : [B*H, C'] --------------------
        cls_ps = [ps_acc.tile([128, 512], f32, tag="ps_acc", name=f"cls_ps{i}") for i in range(2)]
        for cc in range(CCH):
            for half in range(2):
                nc.tensor.matmul(
                    cls_ps[half][:, :],
                    yT_all[:, cc, :],
                    wv_sb[:, cc, half * 512 : (half + 1) * 512],
                    start=(cc == 0),
                    stop=(cc == CCH - 1),
                )
        # copy chunk-wise (alternating engines) so the transposes, diagonal
        # extraction and projection pipeline behind the cls accumulation
        clsn_sb = work.tile([128, C], bf16, tag="scr", name="clsn_sb")
        for m in range(CCH):
            src = cls_ps[m // 4][:, (m % 4) * 128 : (m % 4 + 1) * 128]
            if m % 2 == 0:
                nc.vector.tensor_copy(clsn_sb[:, m * 128 : (m + 1) * 128], src)
            else:
                nc.scalar.copy(clsn_sb[:, m * 128 : (m + 1) * 128], src)
        # transpose each c'-block and keep only the diagonal head block:
        # clsT[p(c'), m, b] with head of c' = 128m + p being 2m + p//64
        clsT = const.tile([128, CCH, BL], bf16, tag="clsT")
        for m in range(CCH):
            pst = ps_t.tile([128, 128], f32, tag="ps_tr")
            nc.tensor.matmul(
                pst[:, :], clsn_sb[:, m * 128 : (m + 1) * 128], ident_bf[:, :]
            )
            pv = pst[:, :].rearrange("p (b h) -> p h b", h=H)
            nc.scalar.copy(clsT[0:64, m, :], pv[0:64, 2 * m, :])
            nc.scalar.copy(clsT[64:128, m, :], pv[64:128, 2 * m + 1, :])

        # ---------------- projection; bias via ones-vector matmul ----------
        out_all = const.tile([BL, C], f32, tag="out_all")
        ps_o = [ps_acc.tile([BL, 512], f32, tag="ps_acc", name=f"ps_o{i}") for i in range(2)]
        for half in range(2):
            nc.tensor.matmul(
                ps_o[half][:, :],
                ones_sb[:, :],
                b_sb[:, half * 512 : (half + 1) * 512],
                start=True,
                stop=False,
            )
        for m in range(CCH):
            for half in range(2):
                nc.tensor.matmul(
                    ps_o[half][:, :],
                    clsT[:, m, :],
                    wp_sb[:, m, half * 512 : (half + 1) * 512],
                    start=False,
                    stop=(m == CCH - 1),
                )
        for half in range(2):
            nc.vector.tensor_copy(
                out_all[:, half * 512 : (half + 1) * 512], ps_o[half][:, :]
            )
        nc.sync.dma_start(out=out_d[:, :], in_=out_all[:, :])

    nc.compile()
    return nc


def get_module():
    if "nc" not in _BUILT:
        _BUILT["nc"] = _build_module()
    return _BUILT["nc"]


def prepare_in_maps(x, W_kv, W_q, W_proj, b_proj):
    """Host-side layout prep: bf16 cast + pre-transpose/swizzle per core."""
    import ml_dtypes

    bf = ml_dtypes.bfloat16
    x = np.asarray(x, dtype=np.float32)

    def swz(w):  # [K, M] -> [128, K/128, M], partition p holds rows kc*128+p
        k, m = w.shape
        return np.ascontiguousarray(
            w.reshape(k // 128, 128, m).transpose(1, 0, 2).astype(bf)
        )

    wkT = swz(np.ascontiguousarray(np.asarray(W_kv)[:, : H * D].T))  # [128,CCH,C]
    wv = swz(np.asarray(W_kv)[:, H * D :])  # [128, CCH, H*D]
    wq = swz(np.asarray(W_q))  # [128, CCH, H*D]
    wp = swz(np.asarray(W_proj))  # [128, CCH, C]
    bp = np.asarray(b_proj, dtype=np.float32).reshape(1, C).astype(bf)

    x_bf = x.astype(bf)  # [B, N, C]
    # x transposed per batch, swizzled: xt[b, p, cc, n] = x[b, n, cc*128+p]
    xt_bf = np.ascontiguousarray(
        x_bf.transpose(0, 2, 1).reshape(B, CCH, 128, N).transpose(0, 2, 1, 3)
    )
    # CLS rows transposed: xclsT[p, cc, b] = x[b, 0, cc*128+p], per core slice
    xcls = np.ascontiguousarray(x_bf[:, 0, :].T.reshape(CCH, 128, B).transpose(1, 0, 2))

    in_maps = []
    for core in range(NCORES):
        sl = slice(core * BL, (core + 1) * BL)
        in_maps.append(
            {
                "x_nat": x_bf[sl],
                "x_tr": xt_bf[sl],
                "xclsT": np.ascontiguousarray(xcls[:, :, sl]),
                "wkT": wkT,
                "wv": wv,
                "wq": wq,
                "wp": wp,
                "b_proj": bp,
            }
        )
    return in_maps


def kernel(x, W_kv, W_q, W_proj, b_proj):
    from concourse.bass_utils import run_bass_kernel_spmd

    nc = get_module()
    in_maps = prepare_in_maps(x, W_kv, W_q, W_proj, b_proj)
    res = run_bass_kernel_spmd(nc, in_maps, core_ids=list(range(NCORES)))
    outs = [res.results[core]["out"] for core in range(NCORES)]
    return np.concatenate(outs, axis=0).reshape(B, 1, C).astype(np.float32)


# revision 30
# speedup vs baseline: 1.1888x; 1.1888x over previous
"""ClassAttention Trainium2 kernel (Bass/Tile), data-parallel over batch on 8 cores.

Math (per batch b):
  q = x[b,0] @ W_q                      -> [H, D]
  k = x[b] @ W_k ; v = x[b] @ W_v       (W_k/W_v = halves of W_kv)
  scores = (q * SCALE) . k  per head    -> [H, N]
  attn = softmax(scores, axis=N)
  cls = attn @ v (per head)             -> [H*D]
  out[b] = cls @ W_proj + b_proj

Algebraic tricks eliminate both giant matmuls (x@W_k and x@W_v):
 1. Fold q into the weights so k is never materialized:
      Q'_b[64h+d, h] = q_b[h,d] * SCALE   (block-diagonal scatter, [C, H])
      G_b = W_k @ Q'_b                    ([C, H], per batch)
      scores^T = G_b^T @ x_b^T
 2. Reassociate the value path: cls = (attn @ x) @ W_v
      y_b = attn_b @ x_b                  ([H, C], contraction over tokens)
      cls  = diag-blocks of (y @ W_v)     (one 128-row matmul for all batches)

All layout work happens on the HOST: x is passed twice (natural and
transposed), both bf16 and pre-swizzled so every DMA is a plain
contiguous copy with 16KB runs per partition. Weights are pre-cast to
bf16 and pre-swizzled too (W_k additionally pre-transposed), so the
device does zero transposes or casts of its inputs. On-chip token index
j = 8p + g (partition p, group g). All matmuls bf16 with fp32
accumulation. Each core handles 8 batches; no collectives.
"""

import numpy as np
from contextlib import ExitStack

B, N, C = 64, 1024, 1024
H, D = 16, 64
SCALE = D**-0.5
NCORES = 8
BL = B // NCORES  # batches per core
CCH = C // 128  # chunks over any 1024-dim
GT = N // 128  # token groups per batch

_BUILT = {}


def _build_module():
    import concourse.mybir as mybir
    import concourse.tile as tile
    from concourse import bacc
    from concourse.masks import make_identity

    f32 = mybir.dt.float32
    bf16 = mybir.dt.bfloat16
    AF = mybir.ActivationFunctionType

    nc = bacc.Bacc("TRN2", target_bir_lowering=False, debug=False)

    x_d = nc.dram_tensor("x_nat", [BL, N, C], bf16, kind="ExternalInput")
    xt_d = nc.dram_tensor("x_tr", [BL, 128, CCH, N], bf16, kind="ExternalInput")
    xclsT_d = nc.dram_tensor("xclsT", [128, CCH, BL], bf16, kind="ExternalInput")
    wkT_d = nc.dram_tensor("wkT", [128, CCH, C], bf16, kind="ExternalInput")
    wv_d = nc.dram_tensor("wv", [128, CCH, H * D], bf16, kind="ExternalInput")
    wq_d = nc.dram_tensor("wq", [128, CCH, H * D], bf16, kind="ExternalInput")
    wp_d = nc.dram_tensor("wp", [128, CCH, C], bf16, kind="ExternalInput")
    bp_d = nc.dram_tensor("b_proj", [1, C], bf16, kind="ExternalInput")
    out_d = nc.dram_tensor("out", [BL, C], f32, kind="ExternalOutput")

    with tile.TileContext(nc) as tc, ExitStack() as ctx:
        const = ctx.enter_context(tc.tile_pool(name="const", bufs=1))
        work = ctx.enter_context(tc.tile_pool(name="work", bufs=2))
        xpool = ctx.enter_context(tc.tile_pool(name="xp", bufs=3))
        xtpool = ctx.enter_context(tc.tile_pool(name="xtp", bufs=3))
        apool = ctx.enter_context(tc.tile_pool(name="ap", bufs=18))
        ps_t = ctx.enter_context(tc.tile_pool(name="ps_t", bufs=2, space="PSUM"))
        ps_acc = ctx.enter_context(tc.tile_pool(name="ps_acc", bufs=6, space="PSUM"))

        # DMA split across both queues so the first weights land fast:
        #   gpsimd (SWDGE): xclsT, wq, x0..x7, wp       (~21 MB)
        #   sync   (HWDGE): b, wkT, xt0..xt7, wv, out   (~21 MB)
        # Issue the first DMA triggers before any compute-engine setup so the
        # queues start streaming immediately.
        xclsT = const.tile([128, CCH, BL], bf16, tag="xclsT")
        nc.gpsimd.dma_start(out=xclsT[:, :, :], in_=xclsT_d[:, :, :])
        wq_sb = const.tile([128, CCH, H * D], bf16, tag="wq")
        nc.gpsimd.dma_start(out=wq_sb[:, :, :], in_=wq_d[:, :, :])
        b_sb = const.tile([1, C], bf16, tag="b")
        nc.sync.dma_start(out=b_sb[:, :], in_=bp_d[:, :])
        wkT_sb = const.tile([128, CCH, C], bf16, tag="wkT")
        nc.sync.dma_start(out=wkT_sb[:, :, :], in_=wkT_d[:, :, :])

        # x/xt arrive in half-tiles so compute can chase the stream at finer
        # granularity (scores/y consume chunks in ascending order).
        def load_x(b):
            x_sb = xpool.tile([128, GT, C], bf16, tag="x")
            xv = x_d[b, :, :].rearrange("(p g) c -> p g c", g=GT)
            nc.gpsimd.dma_start(out=x_sb[:, 0 : GT // 2, :], in_=xv[:, 0 : GT // 2, :])
            nc.gpsimd.dma_start(out=x_sb[:, GT // 2 :, :], in_=xv[:, GT // 2 :, :])
            return x_sb

        def load_xt(b):
            xt_sb = xtpool.tile([128, CCH, N], bf16, tag="xt")
            nc.sync.dma_start(
                out=xt_sb[:, 0 : CCH // 2, :], in_=xt_d[b, :, 0 : CCH // 2, :]
            )
            nc.sync.dma_start(out=xt_sb[:, CCH // 2 :, :], in_=xt_d[b, :, CCH // 2 :, :])
            return xt_sb

        x_tiles = {b: load_x(b) for b in range(2)}
        xt_tiles = {b: load_xt(b) for b in range(2)}

        # ---------------- constants ----------------
        ident_bf = const.tile([128, 128], bf16, tag="ident_bf")
        make_identity(nc, ident_bf[:, :])
        ones_sb = const.tile([1, BL], bf16, tag="ones")
        nc.vector.memset(ones_sb[:, :], 1.0)

        # ---------------- q for all batches (wide form) ----------------
        # psq[b, m] = sum_c xcls[b, c] W_q[c, m]
        psq = [ps_acc.tile([BL, 512], f32, tag="ps_acc", name=f"psq{i}") for i in range(2)]
        for cc in range(CCH):
            for half in range(2):
                nc.tensor.matmul(
                    psq[half][:, :],
                    xclsT[:, cc, :],
                    wq_sb[:, cc, half * 512 : (half + 1) * 512],
                    start=(cc == 0),
                    stop=(cc == CCH - 1),
                )
        qn = work.tile([BL, C], bf16, tag="scr")
        for half in range(2):
            nc.vector.tensor_copy(qn[:, half * 512 : (half + 1) * 512], psq[half][:, :])

        # scatter q into block-diagonal Q' (SCALE folded): Q'[p(j), jc, b*H+h]
        qp_sb = const.tile([128, CCH, BL * H], bf16, tag="qp")
        nc.vector.memset(qp_sb[:, :, :], 0.0)
        for m in range(CCH):
            psqt = ps_t.tile([128, BL], f32, tag="ps_tr")
            nc.tensor.matmul(
                psqt[:, :], qn[:, m * 128 : (m + 1) * 128], ident_bf[0:BL, 0:BL]
            )
            # head of c' = 128*m + p is 2m + p//64
            qv = qp_sb[:, m, :].rearrange("p (b h) -> p h b", h=H)
            nc.scalar.activation(qv[0:64, 2 * m, :], psqt[0:64, :], AF.Copy, scale=SCALE)
            nc.scalar.activation(
                qv[64:128, 2 * m + 1, :], psqt[64:128, :], AF.Copy, scale=SCALE
            )

        # ---------------- G^T = Q'^T @ W_k^T  ([B*H, C], all batches) -------
        gt_ps = [ps_acc.tile([128, 512], f32, tag="ps_acc", name=f"gt_ps{i}") for i in range(2)]
        for jc in range(CCH):
            for half in range(2):
                nc.tensor.matmul(
                    gt_ps[half][:, :],
                    qp_sb[:, jc, :],
                    wkT_sb[:, jc, half * 512 : (half + 1) * 512],
                    start=(jc == 0),
                    stop=(jc == CCH - 1),
                )
        gt_sb = work.tile([128, C], bf16, tag="scr", name="gt_sb")
        for half in range(2):
            nc.vector.tensor_copy(
                gt_sb[:, half * 512 : (half + 1) * 512], gt_ps[half][:, :]
            )
        # transpose to G: g_sb[p(c), cc, b*H+h]
        g_sb = const.tile([128, CCH, BL * H], bf16, tag="g")
        for cb in range(CCH):
            pst = ps_t.tile([128, 128], f32, tag="ps_tr")
            nc.tensor.matmul(
                pst[:, :], gt_sb[:, cb * 128 : (cb + 1) * 128], ident_bf[:, :]
            )
            if cb % 2 == 0:
                nc.vector.tensor_copy(g_sb[:, cb, :], pst[:, :])
            else:
                nc.scalar.copy(g_sb[:, cb, :], pst[:, :])

        # y rows for all batches, assembled at partition b*H+h: [128, C]
        yn_all = const.tile([128, C], bf16, tag="yn_all")
        # y^T for all batches: [p(c), cc, b*H+h]
        yT_all = const.tile([128, CCH, BL * H], bf16, tag="yT")
        yn_pending = {}

        def flush_yn(b):
            # deferred one iteration so the scalar-ring DMA trigger never
            # waits (yn copies are long done), avoiding head-of-line blocking
            # of the exp that shares the scalar engine
            yn = yn_pending.pop(b, None)
            if yn is not None:
                nc.scalar.dma_start(out=yn_all[b * H : (b + 1) * H, :], in_=yn[:, :])

        # ---------------- main loop: software-pipelined ----------------
        # Tensor stream per iteration:
        #   scores_mm(b+1) | attn_tr(b) | y(b) | yT(b)
        # with softmax(b+1) issued on vector/scalar between attn_tr(b) and
        # y(b), so the PE never waits on the softmax serial chain (which
        # would also drop its p-state clock).

        def scores_mm(b):
            """scores^T = G_b^T @ x^T : accumulate into 2 PSUM halves."""
            xt = xt_tiles.pop(b)
            if b + 2 < BL:
                xt_tiles[b + 2] = load_xt(b + 2)
            ps_s = [
                ps_acc.tile([H, 512], f32, tag="ps_acc", name=f"ps_s{i}")
                for i in range(2)
            ]
            for cc in range(CCH):
                for half in range(2):
                    nc.tensor.matmul(
                        ps_s[half][:, :],
                        g_sb[:, cc, b * H : (b + 1) * H],
                        xt[:, cc, half * 512 : (half + 1) * 512],
                        start=(cc == 0),
                        stop=(cc == CCH - 1),
                    )
            return ps_s

        def softmax(ps_s):
            """PSUM scores -> attnT bf16 [H, N] (vector/scalar only).

            Scores are O(1) here (q,k unit-variance, 1/sqrt(D) scale), so the
            usual max-subtraction is skipped: exp reads straight from PSUM
            (fp32 exp overflows only past ~88), halving the serial chain.
            """
            sT_exp = work.tile([H, N], bf16, tag="sTexp")
            sume = []
            for half in range(2):
                acc = work.tile([H, 1], f32, tag=f"sume{half}", name=f"sume{half}")
                nc.scalar.activation(
                    sT_exp[:, half * 512 : (half + 1) * 512],
                    ps_s[half][:, :],
                    AF.Exp,
                    accum_out=acc[:, :],
                )
                sume.append(acc)
            ssum = work.tile([H, 1], f32, tag="ssum")
            nc.vector.tensor_add(ssum[:, :], sume[0][:, :], sume[1][:, :])
            rs = work.tile([H, 1], f32, tag="rs")
            nc.vector.reciprocal(rs[:, :], ssum[:, :])
            attnT = work.tile([H, N], bf16, tag="attnT")
            # out view visits memory g*128+p while reading token 8p+g, so
            # block g of attnT memory holds exactly token-group g's weights
            # in partition order (contiguous stationaries for the transposes)
            nc.vector.tensor_scalar_mul(
                attnT[:, :].rearrange("h (g p) -> h p g", g=GT),
                sT_exp[:, :].rearrange("h (p g) -> h p g", g=GT),
                rs[:, :],
            )
            return attnT

        def attn_tr(attnT):
            """Transpose attn to per-token-group tiles [128, H]."""
            attn_tiles = []
            for g in range(GT):
                ps_a = ps_t.tile([128, H], f32, tag="ps_tr")
                nc.tensor.matmul(
                    ps_a[:, :], attnT[:, g * 128 : (g + 1) * 128], ident_bf[0:H, 0:H]
                )
                a_sb = apool.tile([128, H], bf16, tag="attn")
                nc.vector.tensor_copy(a_sb[:, :], ps_a[:, :])
                attn_tiles.append(a_sb)
            return attn_tiles

        def y_and_yT(b, attn_tiles):
            """y_b = attn_b @ x_b -> yT_all columns."""
            x_sb = x_tiles.pop(b)
            if b + 2 < BL:
                x_tiles[b + 2] = load_x(b + 2)
            ps_y = [
                ps_acc.tile([H, 512], f32, tag="ps_acc", name=f"ps_y{i}")
                for i in range(2)
            ]
            for g in range(GT):
                for half in range(2):
                    nc.tensor.matmul(
                        ps_y[half][:, :],
                        attn_tiles[g][:, :],
                        x_sb[:, g, half * 512 : (half + 1) * 512],
                        start=(g == 0),
                        stop=(g == GT - 1),
                    )
            yn = work.tile([H, C], bf16, tag="yn")
            for half in range(2):
                nc.vector.tensor_copy(
                    yn[:, half * 512 : (half + 1) * 512], ps_y[half][:, :]
                )
            yn_pending[b] = yn

        attnT_cur = softmax(scores_mm(0))
        for b in range(BL):
            flush_yn(b - 1)
            ps_s_next = scores_mm(b + 1) if b + 1 < BL else None
            tiles_b = attn_tr(attnT_cur)
            attnT_cur = softmax(ps_s_next) if ps_s_next is not None else None
            y_and_yT(b, tiles_b)
        flush_yn(BL - 1)

        # late weight loads on the sync queue (needed only below)
        wv_sb = const.tile([128, CCH, H * D], bf16, tag="wv")
        nc.sync.dma_start(out=wv_sb[:, :, :], in_=wv_d[:, :, :])
        wp_sb = const.tile([128, CCH, C], bf16, tag="wp")
        nc.sync.dma_start(out=wp_sb[:, :, :], in_=wp_d[:, :, :])

        # transpose yn_all into yT_all: [p(c), cc, b*H+h]
        for cc in range(CCH):
            ps_yt = ps_t.tile([128, 128], f32, tag="ps_tr", name="ps_yt_end")
            nc.tensor.matmul(
                ps_yt[:, :], yn_all[:, cc * 128 : (cc + 1) * 128], ident_bf[:, :]
            )
            if cc % 2 == 0:
                nc.vector.tensor_copy(yT_all[:, cc, :], ps_yt[:, :])
            else:
                nc.scalar.copy(yT_all[:, cc, :], ps_yt[:, :])

        # ---------------- cls_nat = y @ W_v : [B*H, C'] --------------------
        cls_ps = [ps_acc.tile([128, 512], f32, tag="ps_acc", name=f"cls_ps{i}") for i in range(2)]
        for cc in range(CCH):
            for half in range(2):
                nc.tensor.matmul(
                    cls_ps[half][:, :],
                    yT_all[:, cc, :],
                    wv_sb[:, cc, half * 512 : (half + 1) * 512],
                    start=(cc == 0),
                    stop=(cc == CCH - 1),
                )
        # copy chunk-wise (alternating engines) so the transposes, diagonal
        # extraction and projection pipeline behind the cls accumulation
        clsn_sb = work.tile([128, C], bf16, tag="scr", name="clsn_sb")
        for m in range(CCH):
            src = cls_ps[m // 4][:, (m % 4) * 128 : (m % 4 + 1) * 128]
            if m % 2 == 0:
                nc.vector.tensor_copy(clsn_sb[:, m * 128 : (m + 1) * 128], src)
            else:
                nc.scalar.copy(clsn_sb[:, m * 128 : (m + 1) * 128], src)
        # transpose each c'-block and keep only the diagonal head block:
        # clsT[p(c'), m, b] with head of c' = 128m + p being 2m + p//64
        clsT = const.tile([128, CCH, BL], bf16, tag="clsT")
        for m in range(CCH):
            pst = ps_t.tile([128, 128], f32, tag="ps_tr")
            nc.tensor.matmul(
                pst[:, :], clsn_sb[:, m * 128 : (m + 1) * 128], ident_bf[:, :]
            )
            pv = pst[:, :].rearrange("p (b h) -> p h b", h=H)
            nc.scalar.copy(clsT[0:64, m, :], pv[0:64, 2 * m, :])
            nc.scalar.copy(clsT[64:128, m, :], pv[64:128, 2 * m + 1, :])

        # ---------------- projection; bias via ones-vector matmul ----------
        out_all = const.tile([BL, C], f32, tag="out_all")
        ps_o = [ps_acc.tile([BL, 512], f32, tag="ps_acc", name=f"ps_o{i}") for i in range(2)]
        for half in range(2):
            nc.tensor.matmul(
                ps_o[half][:, :],
                ones_sb[:, :],
                b_sb[:, half * 512 : (half + 1) * 512],
                start=True,
                stop=False,
            )
        for m in range(CCH):
            for half in range(2):
                nc.tensor.matmul(
                    ps_o[half][:, :],
                    clsT[:, m, :],
                    wp_sb[:, m, half * 512 : (half + 1) * 512],
                    start=False,
                    stop=(m == CCH - 1),
                )
        for half in range(2):
            nc.vector.tensor_copy(
                out_all[:, half * 512 : (half + 1) * 512], ps_o[half][:, :]
            )
        nc.sync.dma_start(out=out_d[:, :], in_=out_all[:, :])

    nc.compile()
    return nc


def get_module():
    if "nc" not in _BUILT:
        _BUILT["nc"] = _build_module()
    return _BUILT["nc"]


def prepare_in_maps(x, W_kv, W_q, W_proj, b_proj):
    """Host-side layout prep: bf16 cast + pre-transpose/swizzle per core."""
    import ml_dtypes

    bf = ml_dtypes.bfloat16
    x = np.asarray(x, dtype=np.float32)

    def swz(w):  # [K, M] -> [128, K/128, M], partition p holds rows kc*128+p
        k, m = w.shape
        return np.ascontiguousarray(
            w.reshape(k // 128, 128, m).transpose(1, 0, 2).astype(bf)
        )

    wkT = swz(np.ascontiguousarray(np.asarray(W_kv)[:, : H * D].T))  # [128,CCH,C]
    wv = swz(np.asarray(W_kv)[:, H * D :])  # [128, CCH, H*D]
    wq = swz(np.asarray(W_q))  # [128, CCH, H*D]
    wp = swz(np.asarray(W_proj))  # [128, CCH, C]
    bp = np.asarray(b_proj, dtype=np.float32).reshape(1, C).astype(bf)

    x_bf = x.astype(bf)  # [B, N, C]
    # x transposed per batch, swizzled: xt[b, p, cc, n] = x[b, n, cc*128+p]
    xt_bf = np.ascontiguousarray(
        x_bf.transpose(0, 2, 1).reshape(B, CCH, 128, N).transpose(0, 2, 1, 3)
    )
    # CLS rows transposed: xclsT[p, cc, b] = x[b, 0, cc*128+p], per core slice
    xcls = np.ascontiguousarray(x_bf[:, 0, :].T.reshape(CCH, 128, B).transpose(1, 0, 2))

    in_maps = []
    for core in range(NCORES):
        sl = slice(core * BL, (core + 1) * BL)
        in_maps.append(
            {
                "x_nat": x_bf[sl],
                "x_tr": xt_bf[sl],
                "xclsT": np.ascontiguousarray(xcls[:, :, sl]),
                "wkT": wkT,
                "wv": wv,
                "wq": wq,
                "wp": wp,
                "b_proj": bp,
            }
        )
    return in_maps


def kernel(x, W_kv, W_q, W_proj, b_proj):
    from concourse.bass_utils import run_bass_kernel_spmd

    nc = get_module()
    in_maps = prepare_in_maps(x, W_kv, W_q, W_proj, b_proj)
    res = run_bass_kernel_spmd(nc, in_maps, core_ids=list(range(NCORES)))
    outs = [res.results[core]["out"] for core in range(NCORES)]
    return np.concatenate(outs, axis=0).reshape(B, 1, C).astype(np.float32)
